# revision 16
# baseline (speedup 1.0000x reference)
"""Multi-head attention Bass kernel for Trainium2, SPMD over 8 NeuronCores.

Problem: B=4, S=2048, D=1024, 16 heads x 64. Sharding: core = (batch b, head-group hg)
with b in 0..3, hg in 0..1 -> each core computes 8 heads of one batch.

Per-core pipeline (ScalarE exp is the pacer; everything else hides under it):
  - DMA priority order: weights -> xk -> xq[qb0] -> xv -> xq[rest], so the
    first exp can issue ~16us in while the remaining inputs stream.
  - QKV projections on PE (bf16, fp32 PSUM): KT(hp0) chunk-wise as xk lands,
    QT per-qb just-in-time, V per-kc just-in-time as paced filler closures.
  - Scores S^T[k, q] via row-paired K=64 matmuls (two heads occupy disjoint
    PE row halves -> concurrent on HW).
  - exp on ScalarE from PSUM ([128, 1024] per instruction), scale=1/sqrt(dh),
    bias=-1.5 folded in (keeps exp outputs within fp8e4m3 range), fp8 out.
  - AV with fp8 DoubleRow matmuls: lhsT = [V|1] pairs two 128-row k-chunks
    per instruction (256-deep effective contraction, 2 MACs/cell/cycle).
    The ones column makes PSUM row 64 the softmax denominator for free.
  - Normalize O^T rows by a broadcast reciprocal of the denominator row;
    store O^T per head; the host gather transposes back to [S, heads*dh].
"""
import numpy as np
import ml_dtypes
from contextlib import ExitStack

import concourse.tile as tile
import concourse.mybir as mybir
from concourse import bacc
from concourse.bass_utils import run_bass_kernel_spmd

P = 128
DH = 64
BF = mybir.dt.bfloat16
F32 = mybir.dt.float32
VP = DH + 1       # V row pitch: 64 V columns + the ones (denominator) column
EXP_BIAS = -3.5   # exp(s/8 - 3.5): bounds the bf16 exp range; cancels in
                  # the normalize


def build_attention(S=2048, D=1024, HPC=8, loop_n=1, ablate=(), pbufs=10, pops=3):
    """Build the per-core SPMD program. HPC = heads per core (even)."""
    DC = D // P        # D chunks of 128
    KC = S // P        # k chunks of 128
    NQ = S // 512      # q blocks of 512
    KCP = KC // 2      # kc pairs
    HP = HPC // 2      # head pairs
    CW = HPC * DH      # core output width
    SCALE = 1.0 / float(np.sqrt(DH))
    DR = mybir.MatmulPerfMode.DoubleRow

    nc = bacc.Bacc("TRN2")
    xq = nc.dram_tensor("xq", [DC, P, S], BF, kind="ExternalInput")
    xk = nc.dram_tensor("xk", [DC, P, S], BF, kind="ExternalInput")
    xv = nc.dram_tensor("xv", [DC, P, S], BF, kind="ExternalInput")
    wq = nc.dram_tensor("wq", [DC, P, CW], BF, kind="ExternalInput")
    wk = nc.dram_tensor("wk", [DC, P, CW], BF, kind="ExternalInput")
    wv = nc.dram_tensor("wv", [DC, P, CW], BF, kind="ExternalInput")
    out = nc.dram_tensor("out", [HPC, DH, S], F32, kind="ExternalOutput")

    # exp bias constant (only 0.0/1.0 are pre-registered)
    _bias_t = nc.alloc_sbuf_tensor(f"const-expbias", [128, 1], F32)
    nc.gpsimd.memset(_bias_t.ap(), EXP_BIAS)
    nc.const_aps.aps[(F32, EXP_BIAS)] = _bias_t.ap()
    nc.all_engine_barrier()

    with tile.TileContext(nc) as tc, ExitStack() as ctx:
        xpool = ctx.enter_context(tc.tile_pool(name="x", bufs=1))
        wpool = ctx.enter_context(tc.tile_pool(name="w", bufs=1))
        vpool = ctx.enter_context(tc.tile_pool(name="v", bufs=1))
        qkpool = ctx.enter_context(tc.tile_pool(name="qk", bufs=2))
        ppool = ctx.enter_context(tc.tile_pool(name="p", bufs=pbufs))
        ostag = ctx.enter_context(tc.tile_pool(name="ost", bufs=4))
        outp = ctx.enter_context(tc.tile_pool(name="outp", bufs=4))
        rpool = ctx.enter_context(tc.tile_pool(name="r", bufs=4))
        ps_s = ctx.enter_context(tc.tile_pool(name="ps_s", bufs=1, space="PSUM"))
        ps_o = ctx.enter_context(tc.tile_pool(name="ps_o", bufs=1, space="PSUM"))
        ps_m = ctx.enter_context(tc.tile_pool(name="ps_m", bufs=2, space="PSUM"))

        xs, ws = {}, {}
        vt = None

        def emit_loads():
            # Priority order: wk -> xk (critical path to first exp) ->
            # wq+xq[qb0] -> wv+xv (V-proj for qb0 AV) -> xq rest.
            for name, dram in [("k", wk), ("q", wq), ("v", wv)]:
                ws[name] = wpool.tile([P, DC, CW], BF, tag="w" + name,
                                      name="w" + name)
            for name, dram in [("k", xk), ("q", xq), ("v", xv)]:
                xs[name] = xpool.tile([P, DC, S], BF, tag="x" + name,
                                      name="x" + name + "t")
            nonlocal vt
            # [p(k within chunk), head, kc-pair, pair-slot, padded d] fp8.
            vt = vpool.tile([P, HPC, KCP, 2, VP], BF, tag="V", name="vt")
            nc.any.memset(vt[:, :, :, :, DH : DH + 1], 1.0)

            def ld_w(name, dram):
                nc.sync.dma_start(ws[name][:, :, :], dram[:, :, :].rearrange(
                    "dc p cw -> p dc cw"))

            def ld_x(name, dram, r):
                nc.sync.dma_start(
                    xs[name][:, :, r * 512 : (r + 1) * 512],
                    dram[:, :, r * 512 : (r + 1) * 512].rearrange(
                        "dc p s -> p dc s"))

            ld_w("q", wq)
            ld_x("q", xq, 0)
            ld_w("k", wk)
            ld_x("k", xk, 0)
            ld_x("k", xk, 1)
            ld_w("v", wv)
            ld_x("v", xv, 0)
            ld_x("k", xk, 2)
            ld_x("v", xv, 1)
            ld_x("k", xk, 3)
            ld_x("q", xq, 1)
            ld_x("v", xv, 2)
            ld_x("v", xv, 3)
            ld_x("q", xq, 2)
            ld_x("q", xq, 3)

        qk_done = {}

        def proj_qk_range(t, which, hp, r):
            """t[:, r*512:(r+1)*512] = (W slice)^T x for one 512-col range."""
            qk_done[(which, hp, r)] = True
            pp = ps_m.tile([P, 512], F32, tag="proj", name="pp")
            for dc in range(DC):
                nc.tensor.matmul(
                    pp[:],
                    ws[which][:, dc, hp * P : (hp + 1) * P],
                    xs[which][:, dc, r * 512 : (r + 1) * 512],
                    start=(dc == 0),
                    stop=(dc == DC - 1),
                )
            nc.vector.tensor_copy(t[:, r * 512 : (r + 1) * 512], pp[:])

        def proj_qk_fillers(t, which, hp, ranges):
            """Like proj_qk_range but as closures of 2 accumulating MMs."""
            fillers = []
            for r in ranges:
                state = {}
                qk_done[(which, hp, r)] = False

                def mk(dc0, r=r, state=state):
                    def f():
                        if dc0 == 0:
                            state["pp"] = ps_m.tile([P, 512], F32, tag="proj",
                                                    name="pp")
                        pp = state["pp"]
                        for dc in (dc0, dc0 + 1):
                            nc.tensor.matmul(
                                pp[:],
                                ws[which][:, dc, hp * P : (hp + 1) * P],
                                xs[which][:, dc, r * 512 : (r + 1) * 512],
                                start=(dc == 0),
                                stop=(dc == DC - 1),
                            )
                        if dc0 == DC - 2:
                            nc.vector.tensor_copy(
                                t[:, r * 512 : (r + 1) * 512], pp[:])
                            qk_done[(which, hp, r)] = True
                    return f

                fillers += [mk(d) for d in range(0, DC, 2)]
            return fillers

        v_done = {}

        def proj_v_fillers(kcs):
            """V projection for k-chunks `kcs` as closures of 2 MMs each."""
            fillers = []
            for kc in kcs:
                state = {}
                v_done[kc] = False

                def mk(dc0, kc=kc, state=state):
                    def f():
                        if dc0 == 0:
                            state["pv"] = ps_m.tile([P, 512], F32, tag="proj",
                                                    name="pv")
                        pv = state["pv"]
                        for dc in (dc0, dc0 + 1):
                            nc.tensor.matmul(
                                pv[:],
                                xs["v"][:, dc, kc * P : (kc + 1) * P],
                                ws["v"][:, dc, :],
                                start=(dc == 0),
                                stop=(dc == DC - 1),
                            )
                        if dc0 == DC - 2:
                            nc.vector.tensor_copy(
                                vt[:, :, kc // 2, kc % 2, 0:DH],
                                pv.rearrange("p (h d) -> p h d", d=DH),
                            )
                            v_done[kc] = True
                    return f

                fillers += [mk(d) for d in range(0, DC, 2)]
            return fillers

        def finalize_fillers(osbs, hp, qb):
            """Normalize + store one finished q block (closures)."""
            fillers = []
            for h in (0, 1):
                ch = hp * 2 + h
                osb = osbs[h]
                state = {}

                def rec(osb=osb, state=state):
                    rsb = rpool.tile([1, 512], F32, tag="rc", name="rsb")
                    nc.vector.reciprocal(rsb[:], osb[DH : DH + 1, :])
                    rbc = rpool.tile([DH, 512], F32, tag="rbc", name="rbc")
                    nc.gpsimd.partition_broadcast(rbc[:], rsb[0:1, :])
                    state["rbc"] = rbc

                def norm(ch=ch, qb=qb, osb=osb, state=state):
                    ot = outp.tile([DH, 512], F32, tag="ot", name="ot")
                    nc.vector.tensor_tensor(
                        ot[:], osb[0:DH, :], state["rbc"][:], mybir.AluOpType.mult)
                    nc.sync.dma_start(
                        out[ch, :, qb * 512 : (qb + 1) * 512], ot[:])

                fillers += [rec, norm]
            return fillers

        def attn_block(hp, qb, qt, kt, fillers, fin_q, budget, av_lag=0):
            """Attention for head pair hp, q block qb (512 wide).

            av_lag > 0 defers AV matmul emission by that many kcp slots so
            just-in-time V-projection fillers can land first (hp0 only)."""
            o_ps = [ps_o.tile([DH + 1, 512], F32, tag=f"O{h}", name=f"O{h}")
                    for h in (0, 1)]
            av_q = []

            def emit_av(kcp, pts):
                # Emission-order invariant: the V-proj closures writing
                # vt[kcp] must be emitted before this read, else Tile sees a
                # read of never-written SBUF (no dependency -> race).
                while not (v_done.get(2 * kcp, True)
                           and v_done.get(2 * kcp + 1, True)) and fillers:
                    fillers.pop(0)()
                for h in (0, 1):
                    for j in (0, 1):
                        nc.tensor.matmul(
                            o_ps[h][:],
                            vt[:, hp * 2 + h, kcp, j, 0 : DH + 1],
                            pts[h][:, j, :],
                            start=(kcp == 0 and j == 0),
                            stop=(kcp == KCP - 1 and j == 1),
                        )

            def ensure(key):
                while not qk_done.get(key, True) and fillers:
                    fillers.pop(0)()

            ensure(("q", hp, qb))
            for kcp in range(KCP):
                ensure(("k", hp, kcp // 2))
                s_ps = [ps_s.tile([P, 2, 512], F32, tag=f"S{h}", name=f"S{h}")
                        for h in (0, 1)]
                for j in range(2):
                    kc = 2 * kcp + j
                    for h in (0, 1):
                        nc.tensor.matmul(
                            s_ps[h][:, j, :],
                            kt[h * DH : (h + 1) * DH, kc * P : (kc + 1) * P],
                            qt[h * DH : (h + 1) * DH, qb * 512 : (qb + 1) * 512],
                            start=True,
                            stop=True,
                        )
                pts = []
                for h in (0, 1):
                    pt = ppool.tile([P, 2, 512], BF, tag="pt", name="pt")
                    nc.scalar.activation(
                        pt[:], s_ps[h][:], mybir.ActivationFunctionType.Exp,
                        scale=SCALE, bias=EXP_BIAS,
                    )
                    pts.append(pt)
                av_q.append((kcp, pts))
                if len(av_q) > av_lag:
                    emit_av(*av_q.pop(0))
                if fin_q:
                    fin_q.pop(0)()
                b = budget
                while b and (fillers or fin_q):
                    (fillers or fin_q).pop(0)()
                    b -= 1
            while av_q:
                b = budget
                while b and fillers:
                    fillers.pop(0)()
                    b -= 1
                emit_av(*av_q.pop(0))
            osbs = []
            for h in (0, 1):
                osb = ostag.tile([DH + 1, 512], F32, tag="osb", name="osb")
                nc.vector.tensor_copy(osb[:], o_ps[h][:])
                osbs.append(osb)
            return osbs

        def weave(a, b, ratio):
            """Interleave list b into a: `ratio` items of a per item of b."""
            out_l, ia, ib = [], 0, 0
            while ia < len(a) or ib < len(b):
                out_l += a[ia : ia + ratio]
                ia += ratio
                if ib < len(b):
                    out_l.append(b[ib])
                    ib += 1
            return out_l

        def emit_body():
            emit_loads()
            qt = qkpool.tile([P, S], BF, tag="q", name="qt")
            kt = qkpool.tile([P, S], BF, tag="k", name="kt")
            # QT(hp0, qb0) first: wq + its xq slice are the first DMAs in.
            proj_qk_range(qt, "q", 0, 0)
            # KT(hp0) ranges 0-1 inline (xk r0/r1 land next); r2-3 as the
            # leading fillers (their DMA lands mid-qb0).
            proj_qk_range(kt, "k", 0, 0)
            proj_qk_range(kt, "k", 0, 1)

            fin_q = []
            # hp0 fillers, ordered to match the DMA landing schedule:
            # V kc0-7 (xv r0/r1), KT r2-3, V kc8-11, QT qb1, V kc12-15.
            fillers = (
                proj_v_fillers(range(0, 4))
                + proj_qk_fillers(kt, "k", 0, [2])
                + proj_v_fillers(range(4, 8))
                + proj_qk_fillers(kt, "k", 0, [3])
                + proj_v_fillers(range(8, 12))
                + proj_qk_fillers(qt, "q", 0, [1])
                + proj_v_fillers(range(12, 16))
                + proj_qk_fillers(qt, "q", 0, [2])
                + proj_qk_fillers(qt, "q", 0, [3])
            )

            for hp in range(HP):
                if hp + 1 < HP:
                    qt_next = qkpool.tile([P, S], BF, tag="q", name="qt")
                    kt_next = qkpool.tile([P, S], BF, tag="k", name="kt")
                for qb in range(NQ):
                    if qb == 2 and hp + 1 < HP:
                        # queue next head pair's projections (late: hp0's
                        # early blocks are already PE-saturated with V-proj)
                        fillers += proj_qk_fillers(qt_next, "q", hp + 1,
                                                   range(NQ))
                        fillers += proj_qk_fillers(kt_next, "k", hp + 1,
                                                   range(NQ))
                    budget = 6 if hp == 0 else pops
                    osbs = attn_block(hp, qb, qt, kt, fillers, fin_q, budget,
                                      av_lag=(4 if hp == 0 else 0))
                    fin_q += finalize_fillers(osbs, hp, qb)
                # next head pair's projections must be complete before its
                # attention reads them
                for f in fillers:
                    f()
                fillers = []
                if hp + 1 < HP:
                    qt, kt = qt_next, kt_next
            for f in fin_q:
                f()

        if loop_n > 1:
            with tc.For_i(0, loop_n, 1):
                emit_body()
        else:
            emit_body()

    nc.compile()
    return nc


_NC_CACHE = {}


def _get_nc(S, D, HPC):
    key = (S, D, HPC)
    if key not in _NC_CACHE:
        _NC_CACHE[key] = build_attention(S, D, HPC)
    return _NC_CACHE[key]


def _prep_core_inputs(q_seq, k_seq, v_seq, WQ, WK, WV, b, hg, HPC, D):
    """Host-side shard prep for core (batch b, head group hg)."""
    DC = D // P
    CW = HPC * DH
    bf16 = ml_dtypes.bfloat16

    def xt(x):  # [S, D] -> [DC, P, S] (D-major transpose)
        return np.ascontiguousarray(x.T.reshape(DC, P, -1)).astype(bf16)

    def wslice(w):  # [D, out] -> [DC, P, CW]
        return np.ascontiguousarray(
            w[:, hg * CW : (hg + 1) * CW].reshape(DC, P, CW)
        ).astype(bf16)

    return {
        "xq": xt(q_seq[b]),
        "xk": xt(k_seq[b]),
        "xv": xt(v_seq[b]),
        "wq": wslice(WQ),
        "wk": wslice(WK),
        "wv": wslice(WV),
    }


def kernel(q_seq, k_seq, v_seq, WQ, WK, WV, _trace=False):
    q_seq = np.asarray(q_seq, dtype=np.float32)
    k_seq = np.asarray(k_seq, dtype=np.float32)
    v_seq = np.asarray(v_seq, dtype=np.float32)
    WQ = np.asarray(WQ, dtype=np.float32)
    WK = np.asarray(WK, dtype=np.float32)
    WV = np.asarray(WV, dtype=np.float32)

    B, S, D = q_seq.shape
    NB_HEAD = WQ.shape[1] // DH
    n_cores = 8
    groups_per_batch = n_cores // B          # 2 head groups
    HPC = NB_HEAD // groups_per_batch        # 8 heads per core
    CW = HPC * DH

    nc = _get_nc(S, D, HPC)

    in_maps = []
    for core in range(n_cores):
        b, hg = core // groups_per_batch, core % groups_per_batch
        in_maps.append(_prep_core_inputs(q_seq, k_seq, v_seq, WQ, WK, WV, b, hg, HPC, D))

    res = run_bass_kernel_spmd(
        nc, in_maps, core_ids=list(range(n_cores)), trace=_trace,
        **({"trace_cores": [0], } if _trace else {}),
    )
    if _trace:
        print(f"HW exec time: {res.exec_time_ns} ns")
        if res.instructions_and_trace:
            print("trace:", res.instructions_and_trace[1])

    out = np.empty((B, S, NB_HEAD * DH), dtype=np.float32)
    for core in range(n_cores):
        b, hg = core // groups_per_batch, core % groups_per_batch
        # device output is O^T per head: [HPC, DH, S] -> [S, HPC*DH]
        ot = res.results[core]["out"]
        out[b, :, hg * CW : (hg + 1) * CW] = (
            ot.transpose(2, 0, 1).reshape(S, CW)
        )
    return out
